# revision 22
# baseline (speedup 1.0000x reference)
"""Causal attention block (B=4, S=2048, D=1024, H=16) on 8 Trainium2 NeuronCores.

Sharding: core c = (batch b = c//2, head-group hg = c%2 of 8 heads).
Each core computes QKV projection for its batch restricted to its heads'
columns, causal attention for its 8 heads, and a partial output projection
(its heads' rows of W_proj). Host sums the two partial outputs per batch
pair and returns the full [4, 2048, 1024] result.

v2 design (cost model: matmul = moving-rows only; DVE/Act = free-size only):
  - Everything bf16 (matmul speed identical to fp32r>=256, halves DMA/SBUF,
    removes the fp32r <256-moving-dim penalty so diagonal blocks need no pad).
  - Scores computed transposed sT[k, q] per 2-key-block group into a PSUM
    group tile [128, 2, 512]; ONE exp instruction per full group (Act cost
    is per-free-element, so fewer/larger activations cut the fixed 185ns
    per-instruction overhead); staircase (diagonal) blocks get exact-region
    exps. exp with no max subtraction (scores ~N(0,1), fp32 exp safe).
  - AV restructured: es is the STATIONARY operand (lhsT [128 keys, 128 q])
    and v streams (65 rows incl. a ones column for the denominator), giving
    o[q, v] naturally oriented at ~2x fewer PE rows than streaming scores.
  - Normalization in natural orientation: one batched DVE reciprocal per
    (head, qc) of the 4 denominator columns, then per-qj DVE tensor_scalar
    multiplies (per-partition scalar broadcast; no DRAM-bounce broadcast).
  - Head pairs share an o_norm [128 q, 128] tile (even head cols 0:64, odd
    64:128); one DMA-engine transpose (bf16 xbar, 14ns/tile) per qj lands
    both heads directly into the output-projection lhsT layout. No PE
    transposes, no PSUM->SBUF staging copies.
  - Output projection y = oT.T @ wp in bf16, bias added on DVE, y stored
    bf16 (host upcasts and sums the two partial cores).
  - Software pipelining: attention for qc=0,1 (which only needs tokens
    0:1024 of q/k/v) is interleaved with phase-1 half-1 QKV projection;
    AV/normalize/y work is deferred into later heads' QK/exp slots via a
    budgeted filler queue so the PE never idles while Act runs exps.
"""

from collections import deque

import numpy as np

import concourse.bass as bass
import concourse.mybir as mybir
import concourse.tile as tile
from concourse import bacc
from concourse.bass_utils import run_bass_kernel_spmd
from concourse.masks import make_upper_triangular

F32 = mybir.dt.float32
BF16 = mybir.dt.bfloat16
EMB = 1024
HEADS = 16
HD = 64
B = 4
S = 2048
NCORES = 8
HPC = 8           # heads per core
CD = HPC * HD     # 512 cols per core for each of q/k/v
NKB = S // 128    # 16 key blocks
NQC = S // 512    # 4 query chunks

_EXP = mybir.ActivationFunctionType.Exp


def _build_module():
    nc = bacc.Bacc("TRN2", target_bir_lowering=False, debug=False)
    xT = nc.declare_dram_parameter("xT", [EMB, S], BF16, isOutput=False)
    wq = nc.declare_dram_parameter("wq", [EMB, CD], BF16, isOutput=False)
    wk = nc.declare_dram_parameter("wk", [EMB, CD], BF16, isOutput=False)
    wv = nc.declare_dram_parameter("wv", [EMB, CD], BF16, isOutput=False)
    wp = nc.declare_dram_parameter("wp", [CD, EMB], BF16, isOutput=False)
    bias = nc.declare_dram_parameter("bias", [1, EMB], F32, isOutput=False)
    y = nc.declare_dram_parameter("y", [S, EMB], BF16, isOutput=True)

    with tile.TileContext(nc) as tc:
        _body(tc, nc, xT, wq, wk, wv, wp, bias, y)
    nc.compile()
    return nc


def _body(tc, nc, xT, wq, wk, wv, wp, bias, y):
    from contextlib import ExitStack

    with ExitStack() as ctx:
        persist = ctx.enter_context(tc.tile_pool(name="persist", bufs=1))
        qt = persist.tile([128, 4, S], BF16, tag="qt")
        kt = persist.tile([128, 4, S], BF16, tag="kt")
        vx = persist.tile([128, NKB, HPC, HD + 1], BF16, tag="vx")

        # ones column for denominators (bf16 memset works; f32r did not)
        nc.gpsimd.memset(vx[:, :, :, HD : HD + 1], 1.0)
        # causal mask for diagonal blocks: tri[p, f] = 1.0 iff f >= p
        tri = persist.tile([128, 128], BF16, tag="tri")
        make_upper_triangular(nc, tri[:], val=1.0, diag=True)

        wp_sb = persist.tile([128, 4, EMB], BF16, tag="wp")
        bias_sb = persist.tile([128, 1, EMB], F32, tag="bias")

        # ---------------- pools ----------------
        # PSUM budget (8 banks): qkv 2 + s 2x2 + o 1 + y 1 = 8
        qkv_ps = ctx.enter_context(tc.tile_pool(name="qkvps", bufs=2, space="PSUM"))
        s_pool = ctx.enter_context(tc.tile_pool(name="sps", bufs=2, space="PSUM"))
        o_pool = ctx.enter_context(tc.tile_pool(name="ops", bufs=1, space="PSUM"))
        y_pool = ctx.enter_context(tc.tile_pool(name="yps", bufs=1, space="PSUM"))

        xt_pool = ctx.enter_context(tc.tile_pool(name="xt", bufs=2))
        w_pool = ctx.enter_context(tc.tile_pool(name="w", bufs=4))
        wv_pool = ctx.enter_context(tc.tile_pool(name="wvp", bufs=1))
        # qc3 heads hold 8 es tiles each and the deferred AV of head h pops
        # up to two heads later: keep 3 heads' worth of buffers alive
        es_pool = ctx.enter_context(tc.tile_pool(name="es", bufs=24))
        on_pool = ctx.enter_context(tc.tile_pool(name="onorm", bufs=2))
        oT_pool = ctx.enter_context(tc.tile_pool(name="oT", bufs=2))
        ysb_pool = ctx.enter_context(tc.tile_pool(name="ysb", bufs=2))
        r_pool = ctx.enter_context(tc.tile_pool(name="recip", bufs=2))

        wv_sb = wv_pool.tile([128, 8, CD], BF16, tag="wv")

        # ---------------- phase 1 helpers ----------------
        def ph1_load_xt(half, xt_sb):
            t0 = half * 1024
            for n2 in range(2):
                for kc in range(8):
                    c0 = t0 + n2 * 512
                    # split the startup-gating loads across two queues
                    if half == 1:
                        eng = nc.sync
                    else:
                        eng = nc.gpsimd if (n2 == 0 and kc >= 6) else nc.sync
                    eng.dma_start(
                        out=xt_sb[:, kc, n2 * 512 : (n2 + 1) * 512],
                        in_=xT[kc * 128 : (kc + 1) * 128, c0 : c0 + 512],
                    )

        def ph1_qk_unit(half, xt_sb, wdram, dst, mm, n):
            t0 = half * 1024
            wt = w_pool.tile([128, 8, 128], BF16, tag="w")
            nc.scalar.dma_start(
                out=wt[:],
                in_=wdram[:, mm * 128 : (mm + 1) * 128].rearrange(
                    "(c p) m -> p c m", p=128
                ),
            )
            ps = qkv_ps.tile([128, 512], F32, tag="qkvps")
            for kc in range(8):
                nc.tensor.matmul(
                    ps[:],
                    lhsT=(wt[:, kc, :]),
                    rhs=(xt_sb[:, kc, n * 512 : (n + 1) * 512]),
                    start=(kc == 0),
                    stop=(kc == 7),
                )
            col = t0 + n * 512
            nc.vector.tensor_copy(out=dst[:, mm, col : col + 512], in_=ps[:])

        def ph1_v_unit(half, xt_sb, tc8):
            tg = half * 8 + tc8
            ps = qkv_ps.tile([128, 512], F32, tag="qkvps")
            for kc in range(8):
                nc.tensor.matmul(
                    ps[:],
                    lhsT=(xt_sb[:, kc, tc8 * 128 : (tc8 + 1) * 128]),
                    rhs=(wv_sb[:, kc, :]),
                    start=(kc == 0),
                    stop=(kc == 7),
                )
            nc.vector.tensor_copy(
                out=vx[:, tg, :, 0:HD],
                in_=ps[:].rearrange("p (h d) -> p h d", h=HPC),
            )

        # ---------------- phase 1 DMA staging ----------------
        xt0 = xt_pool.tile([128, 8, 1024], BF16, tag="xt")
        ph1_load_xt(0, xt0)
        # wp/bias are not needed until the first output-projection piece
        nc.gpsimd.dma_start(
            out=wp_sb[:], in_=wp[:].rearrange("(c p) e -> p c e", p=128)
        )
        nc.gpsimd.dma_start(out=bias_sb[:], in_=bias[:].partition_broadcast(128))
        xt1 = xt_pool.tile([128, 8, 1024], BF16, tag="xt")
        ph1_load_xt(1, xt1)

        # half-1 units, paced into qc1/qc2 attention slots. Constraints:
        #  - qk n0 (tokens 1024-1535) and v kb8-11 feed qc2's QK/AV
        #    -> must fully drain during qc1
        #  - qk n1 (tokens 1536-2047) and v kb12-15 feed qc3's QK/AV
        #    -> must fully drain during qc2 (v first: qc3's early AV pops)
        ph1_a = deque()
        ph1_b = deque()
        for wdram, dst in ((wq, qt), (wk, kt)):
            for mm in range(4):
                ph1_a.append(
                    lambda w=wdram, d=dst, m=mm: ph1_qk_unit(1, xt1, w, d, m, 0)
                )
        for tc8 in range(4):
            ph1_a.append(lambda t=tc8: ph1_v_unit(1, xt1, t))
        for tc8 in range(4, 8):
            ph1_b.append(lambda t=tc8: ph1_v_unit(1, xt1, t))
        for wdram, dst in ((wq, qt), (wk, kt)):
            for mm in range(4):
                ph1_b.append(
                    lambda w=wdram, d=dst, m=mm: ph1_qk_unit(1, xt1, w, d, m, 1)
                )

        # ---------------- attention ----------------
        # filler queues: (pe_rows_estimate, emit_fn). AV/norm units are
        # latency-critical (es/o_ps buffer recycling waits on them), so they
        # drain before the bulky output-projection pieces.
        fillers = deque()
        fillers_lo = deque()

        def pop_fillers(budget_rows):
            while fillers and budget_rows > 0:
                rows, fn = fillers.popleft()
                fn()
                budget_rows -= rows
            while fillers_lo and budget_rows > 0:
                rows, fn = fillers_lo.popleft()
                fn()
                budget_rows -= rows

        o_norm_tiles = {}
        # the normalize unit of head h is delayed until head h+1's units are
        # pushed, so by the time it pops the AV it waits on has executed and
        # the in-order DVE queue never blocks phase-1 copies behind it
        pending_norm = [None]

        def make_av_units(h, qc, es_tiles, oT):
            """AV sweep + normalize units for (h, qc). es_tiles[g] holds kb
            (2g, 2g+1). Deferred: they pop during the NEXT head's QK/exp."""
            m, e = h // 2, h % 2
            state = {"oT_tile": oT}

            def av_open():
                state["o_ps"] = o_pool.tile([128, 4, HD + 1], F32, tag="ops", name="o_ps")

            def av_qj(qj):
                o_ps = state["o_ps"]
                kb_last = 4 * qc + qj
                for kb in range(kb_last + 1):
                    g, j = kb // 2, kb % 2
                    nc.tensor.matmul(
                        out=o_ps[:, qj, :],
                        lhsT=(es_tiles[g][:, j, qj * 128 : (qj + 1) * 128]),
                        rhs=(vx[:, kb, h, :]),
                        start=(kb == 0),
                        stop=(kb == kb_last),
                    )

            def av_norm():
                o_ps = state["o_ps"]
                recip = r_pool.tile([128, 4], F32, tag="recip")
                nc.vector.reciprocal(recip[:], o_ps[:, :, HD])
                if e == 0:
                    o_norm_tiles[m] = on_pool.tile([128, 4, 128], BF16, tag="onorm", name="o_norm")
                o_norm = o_norm_tiles[m]
                for qj in range(4):
                    nc.vector.tensor_scalar_mul(
                        o_norm[:, qj, e * HD : (e + 1) * HD],
                        o_ps[:, qj, 0:HD],
                        recip[:, qj : qj + 1],
                    )
                if e == 1:
                    oT = state["oT_tile"]
                    for qj in range(4):
                        nc.sync.dma_start_transpose(
                            out=oT[:, m, qj * 128 : (qj + 1) * 128],
                            in_=o_norm[:, qj, :],
                        )

            def unit01():
                av_open()
                av_qj(0)
                av_qj(1)

            def unit23():
                av_qj(2)
                av_qj(3)

            rows01 = (4 * qc + 1 + 4 * qc + 2) * (HD + 1)
            rows23 = (4 * qc + 3 + 4 * qc + 4) * (HD + 1)
            if pending_norm[0] is not None:
                fillers.append((100, pending_norm[0]))
            fillers.append((rows01, unit01))
            fillers.append((rows23, unit23))
            pending_norm[0] = av_norm

        def flush_norm():
            if pending_norm[0] is not None:
                fillers.append((100, pending_norm[0]))
                pending_norm[0] = None

        def make_y_units(qc, oT):
            def y_piece(tc4, ncol):
                row = qc * 512 + tc4 * 128
                # late chunks alternate two PSUM banks (qkv pool is free by
                # then) so the drain pipeline doesn't serialize on one bank
                if qc >= 2 and (2 * tc4 + ncol) % 2:
                    y_ps = qkv_ps.tile([128, 512], F32, tag="qkvps", name="y_ps")
                else:
                    y_ps = y_pool.tile([128, 512], F32, tag="y", name="y_ps")
                for kc in range(4):
                    nc.tensor.matmul(
                        y_ps[:],
                        lhsT=(oT[:, kc, tc4 * 128 : (tc4 + 1) * 128]),
                        rhs=(wp_sb[:, kc, ncol * 512 : (ncol + 1) * 512]),
                        start=(kc == 0),
                        stop=(kc == 3),
                    )
                y_sb = ysb_pool.tile([128, 512], BF16, tag="ysb")
                nc.vector.tensor_add(
                    y_sb[:],
                    y_ps[:],
                    bias_sb[:, 0, ncol * 512 : (ncol + 1) * 512],
                )
                nc.sync.dma_start(
                    out=y[row : row + 128, ncol * 512 : (ncol + 1) * 512],
                    in_=y_sb[:],
                )

            for tc4 in range(4):
                for ncol in range(2):
                    fillers_lo.append(
                        (2048, lambda t=tc4, n=ncol: y_piece(t, n))
                    )

        # main loop
        slot_state = {"idx": 0}

        def attn_head(h, qc, oT, ph1q=None, every=0):
            m, e = h // 2, h % 2
            po = e * HD
            ngroups = 2 * qc + 2
            es_tiles = []
            for g in range(ngroups):
                s_ps = s_pool.tile([128, 2, 512], F32, tag="s")
                es = es_pool.tile([128, 2, 512], BF16, tag="es")
                es_tiles.append(es)
                nqs = []
                for j in range(2):
                    kb = 2 * g + j
                    r = kb * 128 - qc * 512
                    q0 = max(r, 0)
                    nq = 512 - q0
                    nqs.append((j, kb, r, q0, nq))
                    nc.tensor.matmul(
                        out=s_ps[:, j, q0:512],
                        lhsT=(kt[po : po + HD, m, kb * 128 : (kb + 1) * 128]),
                        rhs=(qt[po : po + HD, m, qc * 512 + q0 : (qc + 1) * 512]),
                        start=True,
                        stop=True,
                    )
                if all(nq == 512 for (_, _, _, _, nq) in nqs):
                    nc.scalar.activation(out=es[:], in_=s_ps[:], func=_EXP)
                else:
                    for j, kb, r, q0, nq in nqs:
                        nc.scalar.activation(
                            out=es[:, j, q0:512],
                            in_=s_ps[:, j, q0:512],
                            func=_EXP,
                        )
                for j, kb, r, q0, nq in nqs:
                    if r >= 0:
                        nc.gpsimd.tensor_mul(
                            es[:, j, q0 : q0 + 128],
                            es[:, j, q0 : q0 + 128],
                            tri[:],
                        )
                # PE filler while the exp chain runs on Act
                pop_fillers(1200)
                slot_state["idx"] += 1
                if ph1q and every and slot_state["idx"] % every == 0:
                    ph1q.popleft()()
            make_av_units(h, qc, es_tiles, oT)

        # qc0/qc1 interleave with phase-1 half-0; their attention needs only
        # tokens 0:512 / 0:1024 which each pair's n-chunk unit just produced
        oT0 = oT_pool.tile([128, 4, 512], BF16, tag="oT")
        for m in range(4):
            ph1_qk_unit(0, xt0, wq, qt, m, 0)
            ph1_qk_unit(0, xt0, wk, kt, m, 0)
            if m == 0:
                # wv load sits behind the first two wt loads on the scalar
                # queue; the v matmuls themselves go between the first two
                # attention heads so the PE isn't gated on it
                for kc in range(8):
                    nc.scalar.dma_start(
                        out=wv_sb[:, kc, :], in_=wv[kc * 128 : (kc + 1) * 128, :]
                    )
            attn_head(2 * m, 0, oT0)
            if m == 0:
                for tc8 in range(4):
                    ph1_v_unit(0, xt0, tc8)
            attn_head(2 * m + 1, 0, oT0)
        flush_norm()
        make_y_units(0, oT0)

        slot_state["idx"] = 0
        oT1 = oT_pool.tile([128, 4, 512], BF16, tag="oT")
        for m in range(4):
            ph1_qk_unit(0, xt0, wq, qt, m, 1)
            ph1_qk_unit(0, xt0, wk, kt, m, 1)
            if m == 0:
                for tc8 in range(4, 8):
                    ph1_v_unit(0, xt0, tc8)
            attn_head(2 * m, 1, oT1, ph1_a, 2)
            attn_head(2 * m + 1, 1, oT1, ph1_a, 2)
        flush_norm()
        make_y_units(1, oT1)
        while ph1_a:
            ph1_a.popleft()()

        slot_state["idx"] = 0
        oT2 = oT_pool.tile([128, 4, 512], BF16, tag="oT")
        for m in range(4):
            attn_head(2 * m, 2, oT2, ph1_b, 3)
            attn_head(2 * m + 1, 2, oT2, ph1_b, 3)
        flush_norm()
        make_y_units(2, oT2)
        while ph1_b:
            ph1_b.popleft()()

        oT3 = oT_pool.tile([128, 4, 512], BF16, tag="oT")
        for m in range(4):
            attn_head(2 * m, 3, oT3)
            attn_head(2 * m + 1, 3, oT3)
        flush_norm()
        make_y_units(3, oT3)

        # drain remaining deferred work
        pop_fillers(10**9)


_MODULE = None


def _get_module():
    global _MODULE
    if _MODULE is None:
        _MODULE = _build_module()
    return _MODULE


def _make_in_maps(x, W_qkv, W_proj, b_proj):
    import ml_dtypes

    bf16 = ml_dtypes.bfloat16
    scale = np.float32(1.0 / np.sqrt(HD))
    bias_half = (np.asarray(b_proj, dtype=np.float32) * 0.5).reshape(1, EMB)
    in_maps = []
    for c in range(NCORES):
        b, hg = c // 2, c % 2
        cols = slice(hg * CD, (hg + 1) * CD)
        in_maps.append(
            {
                "xT": np.ascontiguousarray(
                    np.asarray(x[b], dtype=np.float32).T
                ).astype(bf16),
                "wq": (np.ascontiguousarray(W_qkv[:, 0:EMB][:, cols]) * scale).astype(
                    bf16
                ),
                "wk": np.ascontiguousarray(W_qkv[:, EMB : 2 * EMB][:, cols]).astype(
                    bf16
                ),
                "wv": np.ascontiguousarray(W_qkv[:, 2 * EMB : 3 * EMB][:, cols]).astype(
                    bf16
                ),
                "wp": np.ascontiguousarray(W_proj[cols, :]).astype(bf16),
                "bias": bias_half,
            }
        )
    return in_maps


def kernel(x, W_qkv, W_proj, b_proj, _trace=False, _trace_kwargs=None):
    x = np.asarray(x, dtype=np.float32)
    W_qkv = np.asarray(W_qkv, dtype=np.float32)
    W_proj = np.asarray(W_proj, dtype=np.float32)
    b_proj = np.asarray(b_proj, dtype=np.float32)

    nc = _get_module()
    in_maps = _make_in_maps(x, W_qkv, W_proj, b_proj)
    res = run_bass_kernel_spmd(
        nc, in_maps, list(range(NCORES)), trace=_trace, **(_trace_kwargs or {})
    )
    out = np.empty((B, S, EMB), dtype=np.float32)
    for b in range(B):
        out[b] = res.results[2 * b]["y"].astype(np.float32) + res.results[
            2 * b + 1
        ]["y"].astype(np.float32)
    if _trace:
        return out, res
    return out


# revision 25
# speedup vs baseline: 1.0060x; 1.0060x over previous
"""Causal attention block (B=4, S=2048, D=1024, H=16) on 8 Trainium2 NeuronCores.

Sharding: core c = (batch b = c//2, head-group hg = c%2 of 8 heads).
Each core computes QKV projection for its batch restricted to its heads'
columns, causal attention for its 8 heads, and a partial output projection
(its heads' rows of W_proj). Host sums the two partial outputs per batch
pair and returns the full [4, 2048, 1024] result.

v2 design (cost model: matmul = moving-rows only; DVE/Act = free-size only):
  - Everything bf16 (matmul speed identical to fp32r>=256, halves DMA/SBUF,
    removes the fp32r <256-moving-dim penalty so diagonal blocks need no pad).
  - Scores computed transposed sT[k, q] per 2-key-block group into a PSUM
    group tile [128, 2, 512]; ONE exp instruction per full group (Act cost
    is per-free-element, so fewer/larger activations cut the fixed 185ns
    per-instruction overhead); staircase (diagonal) blocks get exact-region
    exps. exp with no max subtraction (scores ~N(0,1), fp32 exp safe).
  - AV restructured: es is the STATIONARY operand (lhsT [128 keys, 128 q])
    and v streams (65 rows incl. a ones column for the denominator), giving
    o[q, v] naturally oriented at ~2x fewer PE rows than streaming scores.
  - Normalization in natural orientation: one batched DVE reciprocal per
    (head, qc) of the 4 denominator columns, then per-qj DVE tensor_scalar
    multiplies (per-partition scalar broadcast; no DRAM-bounce broadcast).
  - Head pairs share an o_norm [128 q, 128] tile (even head cols 0:64, odd
    64:128); one DMA-engine transpose (bf16 xbar, 14ns/tile) per qj lands
    both heads directly into the output-projection lhsT layout. No PE
    transposes, no PSUM->SBUF staging copies.
  - Output projection y = oT.T @ wp in bf16, bias added on DVE, y stored
    bf16 (host upcasts and sums the two partial cores).
  - Software pipelining: attention for qc=0,1 (which only needs tokens
    0:1024 of q/k/v) is interleaved with phase-1 half-1 QKV projection;
    AV/normalize/y work is deferred into later heads' QK/exp slots via a
    budgeted filler queue so the PE never idles while Act runs exps.
"""

from collections import deque

import numpy as np

import concourse.bass as bass
import concourse.mybir as mybir
import concourse.tile as tile
from concourse import bacc
from concourse.bass_utils import run_bass_kernel_spmd
from concourse.masks import make_upper_triangular

F32 = mybir.dt.float32
BF16 = mybir.dt.bfloat16
EMB = 1024
HEADS = 16
HD = 64
B = 4
S = 2048
NCORES = 8
HPC = 8           # heads per core
CD = HPC * HD     # 512 cols per core for each of q/k/v
NKB = S // 128    # 16 key blocks
NQC = S // 512    # 4 query chunks

_EXP = mybir.ActivationFunctionType.Exp


def _build_module():
    nc = bacc.Bacc("TRN2", target_bir_lowering=False, debug=False)
    xT = nc.declare_dram_parameter("xT", [EMB, S], BF16, isOutput=False)
    wq = nc.declare_dram_parameter("wq", [EMB, CD], BF16, isOutput=False)
    wk = nc.declare_dram_parameter("wk", [EMB, CD], BF16, isOutput=False)
    wv = nc.declare_dram_parameter("wv", [EMB, CD], BF16, isOutput=False)
    wp = nc.declare_dram_parameter("wp", [CD, EMB], BF16, isOutput=False)
    bias = nc.declare_dram_parameter("bias", [1, EMB], F32, isOutput=False)
    y = nc.declare_dram_parameter("y", [S, EMB], BF16, isOutput=True)

    with tile.TileContext(nc) as tc:
        _body(tc, nc, xT, wq, wk, wv, wp, bias, y)
    nc.compile()
    return nc


def _body(tc, nc, xT, wq, wk, wv, wp, bias, y):
    from contextlib import ExitStack

    with ExitStack() as ctx:
        persist = ctx.enter_context(tc.tile_pool(name="persist", bufs=1))
        qt = persist.tile([128, 4, S], BF16, tag="qt")
        kt = persist.tile([128, 4, S], BF16, tag="kt")
        vx = persist.tile([128, NKB, HPC, HD + 1], BF16, tag="vx")

        # ones column for denominators (bf16 memset works; f32r did not)
        nc.gpsimd.memset(vx[:, :, :, HD : HD + 1], 1.0)
        # causal mask for diagonal blocks: tri[p, f] = 1.0 iff f >= p
        tri = persist.tile([128, 128], BF16, tag="tri")
        make_upper_triangular(nc, tri[:], val=1.0, diag=True)

        wp_sb = persist.tile([128, 4, EMB], BF16, tag="wp")
        bias_sb = persist.tile([128, 1, EMB], F32, tag="bias")

        # ---------------- pools ----------------
        # PSUM budget (8 banks): qkv 2 + s 2x2 + o 1 + y 1 = 8
        qkv_ps = ctx.enter_context(tc.tile_pool(name="qkvps", bufs=2, space="PSUM"))
        s_pool = ctx.enter_context(tc.tile_pool(name="sps", bufs=2, space="PSUM"))
        o_pool = ctx.enter_context(tc.tile_pool(name="ops", bufs=1, space="PSUM"))
        y_pool = ctx.enter_context(tc.tile_pool(name="yps", bufs=1, space="PSUM"))

        xt_pool = ctx.enter_context(tc.tile_pool(name="xt", bufs=2))
        w_pool = ctx.enter_context(tc.tile_pool(name="w", bufs=4))
        wv_pool = ctx.enter_context(tc.tile_pool(name="wvp", bufs=1))
        # qc3 heads hold 8 es tiles each and the deferred AV of head h pops
        # up to two heads later: keep 3 heads' worth of buffers alive
        es_pool = ctx.enter_context(tc.tile_pool(name="es", bufs=24))
        on_pool = ctx.enter_context(tc.tile_pool(name="onorm", bufs=2))
        oT_pool = ctx.enter_context(tc.tile_pool(name="oT", bufs=2))
        ysb_pool = ctx.enter_context(tc.tile_pool(name="ysb", bufs=2))
        r_pool = ctx.enter_context(tc.tile_pool(name="recip", bufs=2))

        wv_sb = wv_pool.tile([128, 8, CD], BF16, tag="wv")

        # ---------------- phase 1 helpers ----------------
        def ph1_load_xt(half, xt_sb):
            t0 = half * 1024
            for n2 in range(2):
                for kc in range(8):
                    c0 = t0 + n2 * 512
                    # split the startup-gating loads across two queues
                    if half == 1:
                        eng = nc.sync
                    else:
                        eng = nc.gpsimd if (n2 == 0 and kc >= 6) else nc.sync
                    eng.dma_start(
                        out=xt_sb[:, kc, n2 * 512 : (n2 + 1) * 512],
                        in_=xT[kc * 128 : (kc + 1) * 128, c0 : c0 + 512],
                    )

        def ph1_qk_unit(half, xt_sb, wdram, dst, mm, n):
            t0 = half * 1024
            wt = w_pool.tile([128, 8, 128], BF16, tag="w")
            nc.scalar.dma_start(
                out=wt[:],
                in_=wdram[:, mm * 128 : (mm + 1) * 128].rearrange(
                    "(c p) m -> p c m", p=128
                ),
            )
            ps = qkv_ps.tile([128, 512], F32, tag="qkvps")
            for kc in range(8):
                nc.tensor.matmul(
                    ps[:],
                    lhsT=(wt[:, kc, :]),
                    rhs=(xt_sb[:, kc, n * 512 : (n + 1) * 512]),
                    start=(kc == 0),
                    stop=(kc == 7),
                )
            col = t0 + n * 512
            nc.vector.tensor_copy(out=dst[:, mm, col : col + 512], in_=ps[:])

        def ph1_v_unit(half, xt_sb, tc8):
            tg = half * 8 + tc8
            ps = qkv_ps.tile([128, 512], F32, tag="qkvps")
            for kc in range(8):
                nc.tensor.matmul(
                    ps[:],
                    lhsT=(xt_sb[:, kc, tc8 * 128 : (tc8 + 1) * 128]),
                    rhs=(wv_sb[:, kc, :]),
                    start=(kc == 0),
                    stop=(kc == 7),
                )
            nc.vector.tensor_copy(
                out=vx[:, tg, :, 0:HD],
                in_=ps[:].rearrange("p (h d) -> p h d", h=HPC),
            )

        # ---------------- phase 1 DMA staging ----------------
        xt0 = xt_pool.tile([128, 8, 1024], BF16, tag="xt")
        ph1_load_xt(0, xt0)
        # wp/bias are not needed until the first output-projection piece
        nc.gpsimd.dma_start(
            out=wp_sb[:], in_=wp[:].rearrange("(c p) e -> p c e", p=128)
        )
        nc.gpsimd.dma_start(out=bias_sb[:], in_=bias[:].partition_broadcast(128))
        xt1 = xt_pool.tile([128, 8, 1024], BF16, tag="xt")
        ph1_load_xt(1, xt1)

        # half-1 units, paced into qc1/qc2 attention slots. Constraints:
        #  - qk n0 (tokens 1024-1535) and v kb8-11 feed qc2's QK/AV
        #    -> must fully drain during qc1
        #  - v kb12-15 feed qc3's AV -> drain during qc2
        #  - qk n1 (tokens 1536-2047) feed qc3's QK -> emitted eagerly at the
        #    top of each qc3 pair (the PE there is otherwise Act-bound)
        ph1_a = deque()
        ph1_b = deque()
        for wdram, dst in ((wq, qt), (wk, kt)):
            for mm in range(4):
                ph1_a.append(
                    lambda w=wdram, d=dst, m=mm: ph1_qk_unit(1, xt1, w, d, m, 0)
                )
        for tc8 in range(4):
            ph1_a.append(lambda t=tc8: ph1_v_unit(1, xt1, t))
        for tc8 in range(4, 8):
            ph1_b.append(lambda t=tc8: ph1_v_unit(1, xt1, t))

        # ---------------- attention ----------------
        # filler queues: (pe_rows_estimate, emit_fn). AV/norm units are
        # latency-critical (es/o_ps buffer recycling waits on them), so they
        # drain before the bulky output-projection pieces.
        fillers = deque()
        fillers_lo = deque()

        def pop_fillers(budget_rows):
            while fillers and budget_rows > 0:
                rows, fn = fillers.popleft()
                fn()
                budget_rows -= rows
            while fillers_lo and budget_rows > 0:
                rows, fn = fillers_lo.popleft()
                fn()
                budget_rows -= rows

        o_norm_tiles = {}
        # the normalize unit of head h is delayed until head h+1's units are
        # pushed, so by the time it pops the AV it waits on has executed and
        # the in-order DVE queue never blocks phase-1 copies behind it
        pending_norm = [None]

        def make_av_units(h, qc, es_tiles, oT):
            """AV sweep + normalize units for (h, qc). es_tiles[g] holds kb
            (2g, 2g+1). Deferred: they pop during the NEXT head's QK/exp."""
            m, e = h // 2, h % 2
            state = {"oT_tile": oT}

            def av_open():
                state["o_ps"] = o_pool.tile([128, 4, HD + 1], F32, tag="ops", name="o_ps")

            def av_qj(qj):
                o_ps = state["o_ps"]
                kb_last = 4 * qc + qj
                for kb in range(kb_last + 1):
                    g, j = kb // 2, kb % 2
                    nc.tensor.matmul(
                        out=o_ps[:, qj, :],
                        lhsT=(es_tiles[g][:, j, qj * 128 : (qj + 1) * 128]),
                        rhs=(vx[:, kb, h, :]),
                        start=(kb == 0),
                        stop=(kb == kb_last),
                    )

            def av_norm():
                o_ps = state["o_ps"]
                recip = r_pool.tile([128, 4], F32, tag="recip")
                nc.vector.reciprocal(recip[:], o_ps[:, :, HD])
                if e == 0:
                    o_norm_tiles[m] = on_pool.tile([128, 4, 128], BF16, tag="onorm", name="o_norm")
                o_norm = o_norm_tiles[m]
                for qj in range(4):
                    nc.vector.tensor_scalar_mul(
                        o_norm[:, qj, e * HD : (e + 1) * HD],
                        o_ps[:, qj, 0:HD],
                        recip[:, qj : qj + 1],
                    )
                if e == 1:
                    oT = state["oT_tile"]
                    for qj in range(4):
                        nc.sync.dma_start_transpose(
                            out=oT[:, m, qj * 128 : (qj + 1) * 128],
                            in_=o_norm[:, qj, :],
                        )

            def unit01():
                av_open()
                av_qj(0)
                av_qj(1)

            def unit23():
                av_qj(2)
                av_qj(3)

            rows01 = (4 * qc + 1 + 4 * qc + 2) * (HD + 1)
            rows23 = (4 * qc + 3 + 4 * qc + 4) * (HD + 1)
            if pending_norm[0] is not None:
                fillers.append((100, pending_norm[0]))
            fillers.append((rows01, unit01))
            fillers.append((rows23, unit23))
            pending_norm[0] = av_norm

        def flush_norm():
            if pending_norm[0] is not None:
                fillers.append((100, pending_norm[0]))
                pending_norm[0] = None

        def make_y_units(qc, oT):
            def y_piece(tc4, ncol):
                row = qc * 512 + tc4 * 128
                # late chunks alternate two PSUM banks (qkv pool is free by
                # then) so the drain pipeline doesn't serialize on one bank
                if qc >= 2 and (2 * tc4 + ncol) % 2:
                    y_ps = qkv_ps.tile([128, 512], F32, tag="qkvps", name="y_ps")
                else:
                    y_ps = y_pool.tile([128, 512], F32, tag="y", name="y_ps")
                for kc in range(4):
                    nc.tensor.matmul(
                        y_ps[:],
                        lhsT=(oT[:, kc, tc4 * 128 : (tc4 + 1) * 128]),
                        rhs=(wp_sb[:, kc, ncol * 512 : (ncol + 1) * 512]),
                        start=(kc == 0),
                        stop=(kc == 3),
                    )
                y_sb = ysb_pool.tile([128, 512], BF16, tag="ysb")
                nc.vector.tensor_add(
                    y_sb[:],
                    y_ps[:],
                    bias_sb[:, 0, ncol * 512 : (ncol + 1) * 512],
                )
                nc.sync.dma_start(
                    out=y[row : row + 128, ncol * 512 : (ncol + 1) * 512],
                    in_=y_sb[:],
                )

            for tc4 in range(4):
                for ncol in range(2):
                    fillers_lo.append(
                        (2048, lambda t=tc4, n=ncol: y_piece(t, n))
                    )

        # main loop
        slot_state = {"idx": 0}

        def attn_head(h, qc, oT, ph1q=None, every=0):
            m, e = h // 2, h % 2
            po = e * HD
            ngroups = 2 * qc + 2
            es_tiles = []
            for g in range(ngroups):
                s_ps = s_pool.tile([128, 2, 512], F32, tag="s")
                es = es_pool.tile([128, 2, 512], BF16, tag="es")
                es_tiles.append(es)
                nqs = []
                for j in range(2):
                    kb = 2 * g + j
                    r = kb * 128 - qc * 512
                    q0 = max(r, 0)
                    nq = 512 - q0
                    nqs.append((j, kb, r, q0, nq))
                    nc.tensor.matmul(
                        out=s_ps[:, j, q0:512],
                        lhsT=(kt[po : po + HD, m, kb * 128 : (kb + 1) * 128]),
                        rhs=(qt[po : po + HD, m, qc * 512 + q0 : (qc + 1) * 512]),
                        start=True,
                        stop=True,
                    )
                if all(nq == 512 for (_, _, _, _, nq) in nqs):
                    nc.scalar.activation(out=es[:], in_=s_ps[:], func=_EXP)
                else:
                    for j, kb, r, q0, nq in nqs:
                        nc.scalar.activation(
                            out=es[:, j, q0:512],
                            in_=s_ps[:, j, q0:512],
                            func=_EXP,
                        )
                for j, kb, r, q0, nq in nqs:
                    if r >= 0:
                        nc.gpsimd.tensor_mul(
                            es[:, j, q0 : q0 + 128],
                            es[:, j, q0 : q0 + 128],
                            tri[:],
                        )
                # PE filler while the exp chain runs on Act
                pop_fillers(2500)
                slot_state["idx"] += 1
                if ph1q and every and slot_state["idx"] % every == 0:
                    ph1q.popleft()()
            make_av_units(h, qc, es_tiles, oT)

        # qc0/qc1 interleave with phase-1 half-0; their attention needs only
        # tokens 0:512 / 0:1024 which each pair's n-chunk unit just produced
        oT0 = oT_pool.tile([128, 4, 512], BF16, tag="oT")
        for m in range(4):
            ph1_qk_unit(0, xt0, wq, qt, m, 0)
            ph1_qk_unit(0, xt0, wk, kt, m, 0)
            if m == 0:
                # wv load sits behind the first two wt loads on the scalar
                # queue; the v matmuls themselves go between the first two
                # attention heads so the PE isn't gated on it
                for kc in range(8):
                    nc.scalar.dma_start(
                        out=wv_sb[:, kc, :], in_=wv[kc * 128 : (kc + 1) * 128, :]
                    )
            attn_head(2 * m, 0, oT0)
            if m == 0:
                for tc8 in range(4):
                    ph1_v_unit(0, xt0, tc8)
            attn_head(2 * m + 1, 0, oT0)
        flush_norm()
        make_y_units(0, oT0)

        slot_state["idx"] = 0
        oT1 = oT_pool.tile([128, 4, 512], BF16, tag="oT")
        for m in range(4):
            ph1_qk_unit(0, xt0, wq, qt, m, 1)
            ph1_qk_unit(0, xt0, wk, kt, m, 1)
            if m == 0:
                for tc8 in range(4, 8):
                    ph1_v_unit(0, xt0, tc8)
            attn_head(2 * m, 1, oT1, ph1_a, 2)
            attn_head(2 * m + 1, 1, oT1, ph1_a, 2)
        flush_norm()
        make_y_units(1, oT1)
        while ph1_a:
            ph1_a.popleft()()

        slot_state["idx"] = 0
        oT2 = oT_pool.tile([128, 4, 512], BF16, tag="oT")
        for m in range(4):
            attn_head(2 * m, 2, oT2, ph1_b, 8)
            attn_head(2 * m + 1, 2, oT2, ph1_b, 8)
        flush_norm()
        make_y_units(2, oT2)
        while ph1_b:
            ph1_b.popleft()()

        oT3 = oT_pool.tile([128, 4, 512], BF16, tag="oT")
        for m in range(4):
            ph1_qk_unit(1, xt1, wq, qt, m, 1)
            ph1_qk_unit(1, xt1, wk, kt, m, 1)
            attn_head(2 * m, 3, oT3)
            attn_head(2 * m + 1, 3, oT3)
        flush_norm()
        make_y_units(3, oT3)

        # drain remaining deferred work
        pop_fillers(10**9)


_MODULE = None


def _get_module():
    global _MODULE
    if _MODULE is None:
        _MODULE = _build_module()
    return _MODULE


def _make_in_maps(x, W_qkv, W_proj, b_proj):
    import ml_dtypes

    bf16 = ml_dtypes.bfloat16
    scale = np.float32(1.0 / np.sqrt(HD))
    bias_half = (np.asarray(b_proj, dtype=np.float32) * 0.5).reshape(1, EMB)
    in_maps = []
    for c in range(NCORES):
        b, hg = c // 2, c % 2
        cols = slice(hg * CD, (hg + 1) * CD)
        in_maps.append(
            {
                "xT": np.ascontiguousarray(
                    np.asarray(x[b], dtype=np.float32).T
                ).astype(bf16),
                "wq": (np.ascontiguousarray(W_qkv[:, 0:EMB][:, cols]) * scale).astype(
                    bf16
                ),
                "wk": np.ascontiguousarray(W_qkv[:, EMB : 2 * EMB][:, cols]).astype(
                    bf16
                ),
                "wv": np.ascontiguousarray(W_qkv[:, 2 * EMB : 3 * EMB][:, cols]).astype(
                    bf16
                ),
                "wp": np.ascontiguousarray(W_proj[cols, :]).astype(bf16),
                "bias": bias_half,
            }
        )
    return in_maps


def kernel(x, W_qkv, W_proj, b_proj, _trace=False, _trace_kwargs=None):
    x = np.asarray(x, dtype=np.float32)
    W_qkv = np.asarray(W_qkv, dtype=np.float32)
    W_proj = np.asarray(W_proj, dtype=np.float32)
    b_proj = np.asarray(b_proj, dtype=np.float32)

    nc = _get_module()
    in_maps = _make_in_maps(x, W_qkv, W_proj, b_proj)
    res = run_bass_kernel_spmd(
        nc, in_maps, list(range(NCORES)), trace=_trace, **(_trace_kwargs or {})
    )
    out = np.empty((B, S, EMB), dtype=np.float32)
    for b in range(B):
        out[b] = res.results[2 * b]["y"].astype(np.float32) + res.results[
            2 * b + 1
        ]["y"].astype(np.float32)
    if _trace:
        return out, res
    return out


# revision 26
# speedup vs baseline: 1.0791x; 1.0728x over previous
"""Causal attention block (B=4, S=2048, D=1024, H=16) on 8 Trainium2 NeuronCores.

Sharding: core c = (batch b = c//2, head-group hg = c%2 of 8 heads).
Each core computes QKV projection for its batch restricted to its heads'
columns, causal attention for its 8 heads, and a partial output projection
(its heads' rows of W_proj). Host sums the two partial outputs per batch
pair and returns the full [4, 2048, 1024] result.

v2 design (cost model: matmul = moving-rows only; DVE/Act = free-size only):
  - Everything bf16 (matmul speed identical to fp32r>=256, halves DMA/SBUF,
    removes the fp32r <256-moving-dim penalty so diagonal blocks need no pad).
  - Scores computed transposed sT[k, q] per 2-key-block group into a PSUM
    group tile [128, 2, 512]; ONE exp instruction per full group (Act cost
    is per-free-element, so fewer/larger activations cut the fixed 185ns
    per-instruction overhead); staircase (diagonal) blocks get exact-region
    exps. exp with no max subtraction (scores ~N(0,1), fp32 exp safe).
  - AV restructured: es is the STATIONARY operand (lhsT [128 keys, 128 q])
    and v streams (65 rows incl. a ones column for the denominator), giving
    o[q, v] naturally oriented at ~2x fewer PE rows than streaming scores.
  - Normalization in natural orientation: one batched DVE reciprocal per
    (head, qc) of the 4 denominator columns, then per-qj DVE tensor_scalar
    multiplies (per-partition scalar broadcast; no DRAM-bounce broadcast).
  - Head pairs share an o_norm [128 q, 128] tile (even head cols 0:64, odd
    64:128); one DMA-engine transpose (bf16 xbar, 14ns/tile) per qj lands
    both heads directly into the output-projection lhsT layout. No PE
    transposes, no PSUM->SBUF staging copies.
  - Output projection y = oT.T @ wp in bf16, bias added on DVE, y stored
    bf16 (host upcasts and sums the two partial cores).
  - Software pipelining: attention for qc=0,1 (which only needs tokens
    0:1024 of q/k/v) is interleaved with phase-1 half-1 QKV projection;
    AV/normalize/y work is deferred into later heads' QK/exp slots via a
    budgeted filler queue so the PE never idles while Act runs exps.
"""

from collections import deque

import numpy as np

import concourse.bass as bass
import concourse.mybir as mybir
import concourse.tile as tile
from concourse import bacc
from concourse.bass_utils import run_bass_kernel_spmd
from concourse.masks import make_upper_triangular

F32 = mybir.dt.float32
BF16 = mybir.dt.bfloat16
EMB = 1024
HEADS = 16
HD = 64
B = 4
S = 2048
NCORES = 8
HPC = 8           # heads per core
CD = HPC * HD     # 512 cols per core for each of q/k/v
NKB = S // 128    # 16 key blocks
NQC = S // 512    # 4 query chunks

_EXP = mybir.ActivationFunctionType.Exp


def _build_module():
    nc = bacc.Bacc("TRN2", target_bir_lowering=False, debug=False)
    xT = nc.declare_dram_parameter("xT", [EMB, S], BF16, isOutput=False)
    # wq/wk host-rearranged to [p, mtile, chunk, m] so each mtile loads as
    # one contiguous 2KB-per-partition DMA (128 descriptors, not 1024)
    wq = nc.declare_dram_parameter("wq", [128, 4, 8, 128], BF16, isOutput=False)
    wk = nc.declare_dram_parameter("wk", [128, 4, 8, 128], BF16, isOutput=False)
    wv = nc.declare_dram_parameter("wv", [EMB, CD], BF16, isOutput=False)
    wp = nc.declare_dram_parameter("wp", [CD, EMB], BF16, isOutput=False)
    bias = nc.declare_dram_parameter("bias", [1, EMB], F32, isOutput=False)
    y = nc.declare_dram_parameter("y", [S, EMB], BF16, isOutput=True)

    with tile.TileContext(nc) as tc:
        _body(tc, nc, xT, wq, wk, wv, wp, bias, y)
    nc.compile()
    return nc


def _body(tc, nc, xT, wq, wk, wv, wp, bias, y):
    from contextlib import ExitStack

    with ExitStack() as ctx:
        persist = ctx.enter_context(tc.tile_pool(name="persist", bufs=1))
        qt = persist.tile([128, 4, S], BF16, tag="qt")
        kt = persist.tile([128, 4, S], BF16, tag="kt")
        vx = persist.tile([128, NKB, HPC, HD + 1], BF16, tag="vx")

        # ones column for denominators (bf16 memset works; f32r did not)
        nc.gpsimd.memset(vx[:, :, :, HD : HD + 1], 1.0)
        # causal mask for diagonal blocks: tri[p, f] = 1.0 iff f >= p
        tri = persist.tile([128, 128], BF16, tag="tri")
        make_upper_triangular(nc, tri[:], val=1.0, diag=True)

        wp_sb = persist.tile([128, 4, EMB], BF16, tag="wp")
        bias_sb = persist.tile([128, 1, EMB], F32, tag="bias")
        wq_sb = persist.tile([128, 4, 8, 128], BF16, tag="wq")
        wk_sb = persist.tile([128, 4, 8, 128], BF16, tag="wk")
        for mm in range(4):
            nc.scalar.dma_start(out=wq_sb[:, mm], in_=wq[:, mm])
            nc.scalar.dma_start(out=wk_sb[:, mm], in_=wk[:, mm])

        # ---------------- pools ----------------
        # PSUM budget (8 banks): qkv 2 + s 2x2 + o 1 + y 1 = 8
        qkv_ps = ctx.enter_context(tc.tile_pool(name="qkvps", bufs=2, space="PSUM"))
        s_pool = ctx.enter_context(tc.tile_pool(name="sps", bufs=2, space="PSUM"))
        o_pool = ctx.enter_context(tc.tile_pool(name="ops", bufs=1, space="PSUM"))
        y_pool = ctx.enter_context(tc.tile_pool(name="yps", bufs=1, space="PSUM"))

        xt_pool = ctx.enter_context(tc.tile_pool(name="xt", bufs=2))
        wv_pool = ctx.enter_context(tc.tile_pool(name="wvp", bufs=1))
        # qc3 heads hold 8 es tiles each and the deferred AV of head h pops
        # up to two heads later: keep 3 heads' worth of buffers alive
        es_pool = ctx.enter_context(tc.tile_pool(name="es", bufs=24))
        on_pool = ctx.enter_context(tc.tile_pool(name="onorm", bufs=2))
        oT_pool = ctx.enter_context(tc.tile_pool(name="oT", bufs=2))
        ysb_pool = ctx.enter_context(tc.tile_pool(name="ysb", bufs=2))
        r_pool = ctx.enter_context(tc.tile_pool(name="recip", bufs=2))

        wv_sb = wv_pool.tile([128, 8, CD], BF16, tag="wv")

        # ---------------- phase 1 helpers ----------------
        def ph1_load_xt(half, xt_sb):
            t0 = half * 1024
            for n2 in range(2):
                for kc in range(8):
                    c0 = t0 + n2 * 512
                    # split the startup-gating loads across two queues
                    if half == 1:
                        eng = nc.sync
                    else:
                        eng = nc.gpsimd if (n2 == 0 and kc >= 6) else nc.sync
                    eng.dma_start(
                        out=xt_sb[:, kc, n2 * 512 : (n2 + 1) * 512],
                        in_=xT[kc * 128 : (kc + 1) * 128, c0 : c0 + 512],
                    )

        def ph1_qk_unit(half, xt_sb, wdram, dst, mm, n):
            t0 = half * 1024
            wt = wq_sb if wdram is wq else wk_sb
            ps = qkv_ps.tile([128, 512], F32, tag="qkvps")
            for kc in range(8):
                nc.tensor.matmul(
                    ps[:],
                    lhsT=(wt[:, mm, kc, :]),
                    rhs=(xt_sb[:, kc, n * 512 : (n + 1) * 512]),
                    start=(kc == 0),
                    stop=(kc == 7),
                )
            col = t0 + n * 512
            nc.vector.tensor_copy(out=dst[:, mm, col : col + 512], in_=ps[:])

        def ph1_v_unit(half, xt_sb, tc8):
            tg = half * 8 + tc8
            ps = qkv_ps.tile([128, 512], F32, tag="qkvps")
            for kc in range(8):
                nc.tensor.matmul(
                    ps[:],
                    lhsT=(xt_sb[:, kc, tc8 * 128 : (tc8 + 1) * 128]),
                    rhs=(wv_sb[:, kc, :]),
                    start=(kc == 0),
                    stop=(kc == 7),
                )
            nc.vector.tensor_copy(
                out=vx[:, tg, :, 0:HD],
                in_=ps[:].rearrange("p (h d) -> p h d", h=HPC),
            )

        # ---------------- phase 1 DMA staging ----------------
        xt0 = xt_pool.tile([128, 8, 1024], BF16, tag="xt")
        ph1_load_xt(0, xt0)
        # wp/bias are not needed until the first output-projection piece
        nc.gpsimd.dma_start(
            out=wp_sb[:], in_=wp[:].rearrange("(c p) e -> p c e", p=128)
        )
        nc.gpsimd.dma_start(out=bias_sb[:], in_=bias[:].partition_broadcast(128))
        xt1 = xt_pool.tile([128, 8, 1024], BF16, tag="xt")
        ph1_load_xt(1, xt1)

        # half-1 units, paced into qc1/qc2 attention slots. Constraints:
        #  - qk n0 (tokens 1024-1535) and v kb8-11 feed qc2's QK/AV
        #    -> must fully drain during qc1
        #  - v kb12-15 feed qc3's AV -> drain during qc2
        #  - qk n1 (tokens 1536-2047) feed qc3's QK -> emitted eagerly at the
        #    top of each qc3 pair (the PE there is otherwise Act-bound)
        ph1_a = deque()
        ph1_b = deque()
        for wdram, dst in ((wq, qt), (wk, kt)):
            for mm in range(4):
                ph1_a.append(
                    lambda w=wdram, d=dst, m=mm: ph1_qk_unit(1, xt1, w, d, m, 0)
                )
        for tc8 in range(4):
            ph1_a.append(lambda t=tc8: ph1_v_unit(1, xt1, t))
        for tc8 in range(4, 8):
            ph1_b.append(lambda t=tc8: ph1_v_unit(1, xt1, t))

        # ---------------- attention ----------------
        # filler queues: (pe_rows_estimate, emit_fn). AV/norm units are
        # latency-critical (es/o_ps buffer recycling waits on them), so they
        # drain before the bulky output-projection pieces.
        fillers = deque()
        fillers_lo = deque()

        def pop_fillers(budget_rows):
            while fillers and budget_rows > 0:
                rows, fn = fillers.popleft()
                fn()
                budget_rows -= rows
            while fillers_lo and budget_rows > 0:
                rows, fn = fillers_lo.popleft()
                fn()
                budget_rows -= rows

        o_norm_tiles = {}
        # the normalize unit of head h is delayed until head h+1's units are
        # pushed, so by the time it pops the AV it waits on has executed and
        # the in-order DVE queue never blocks phase-1 copies behind it
        pending_norm = [None]

        def make_av_units(h, qc, es_tiles, oT):
            """AV sweep + normalize units for (h, qc). es_tiles[g] holds kb
            (2g, 2g+1). Deferred: they pop during the NEXT head's QK/exp."""
            m, e = h // 2, h % 2
            state = {"oT_tile": oT}

            def av_open():
                state["o_ps"] = o_pool.tile([128, 4, HD + 1], F32, tag="ops", name="o_ps")

            def av_qj(qj):
                o_ps = state["o_ps"]
                kb_last = 4 * qc + qj
                for kb in range(kb_last + 1):
                    g, j = kb // 2, kb % 2
                    nc.tensor.matmul(
                        out=o_ps[:, qj, :],
                        lhsT=(es_tiles[g][:, j, qj * 128 : (qj + 1) * 128]),
                        rhs=(vx[:, kb, h, :]),
                        start=(kb == 0),
                        stop=(kb == kb_last),
                    )

            def av_norm():
                o_ps = state["o_ps"]
                recip = r_pool.tile([128, 4], F32, tag="recip")
                nc.vector.reciprocal(recip[:], o_ps[:, :, HD])
                if e == 0:
                    o_norm_tiles[m] = on_pool.tile([128, 4, 128], BF16, tag="onorm", name="o_norm")
                o_norm = o_norm_tiles[m]
                for qj in range(4):
                    nc.vector.tensor_scalar_mul(
                        o_norm[:, qj, e * HD : (e + 1) * HD],
                        o_ps[:, qj, 0:HD],
                        recip[:, qj : qj + 1],
                    )
                if e == 1:
                    oT = state["oT_tile"]
                    for qj in range(4):
                        nc.sync.dma_start_transpose(
                            out=oT[:, m, qj * 128 : (qj + 1) * 128],
                            in_=o_norm[:, qj, :],
                        )

            def unit01():
                av_open()
                av_qj(0)
                av_qj(1)

            def unit23():
                av_qj(2)
                av_qj(3)

            rows01 = (4 * qc + 1 + 4 * qc + 2) * (HD + 1)
            rows23 = (4 * qc + 3 + 4 * qc + 4) * (HD + 1)
            if pending_norm[0] is not None:
                fillers.append((100, pending_norm[0]))
            fillers.append((rows01, unit01))
            fillers.append((rows23, unit23))
            pending_norm[0] = av_norm

        def flush_norm():
            if pending_norm[0] is not None:
                fillers.append((100, pending_norm[0]))
                pending_norm[0] = None

        def make_y_units(qc, oT):
            def y_piece(tc4, ncol):
                row = qc * 512 + tc4 * 128
                # late chunks alternate two PSUM banks (qkv pool is free by
                # then) so the drain pipeline doesn't serialize on one bank
                if qc >= 2 and (2 * tc4 + ncol) % 2:
                    y_ps = qkv_ps.tile([128, 512], F32, tag="qkvps", name="y_ps")
                else:
                    y_ps = y_pool.tile([128, 512], F32, tag="y", name="y_ps")
                for kc in range(4):
                    nc.tensor.matmul(
                        y_ps[:],
                        lhsT=(oT[:, kc, tc4 * 128 : (tc4 + 1) * 128]),
                        rhs=(wp_sb[:, kc, ncol * 512 : (ncol + 1) * 512]),
                        start=(kc == 0),
                        stop=(kc == 3),
                    )
                y_sb = ysb_pool.tile([128, 512], BF16, tag="ysb")
                nc.vector.tensor_add(
                    y_sb[:],
                    y_ps[:],
                    bias_sb[:, 0, ncol * 512 : (ncol + 1) * 512],
                )
                nc.sync.dma_start(
                    out=y[row : row + 128, ncol * 512 : (ncol + 1) * 512],
                    in_=y_sb[:],
                )

            for tc4 in range(4):
                for ncol in range(2):
                    fillers_lo.append(
                        (2048, lambda t=tc4, n=ncol: y_piece(t, n))
                    )

        # main loop
        slot_state = {"idx": 0}

        def attn_head(h, qc, oT, ph1q=None, every=0):
            m, e = h // 2, h % 2
            po = e * HD
            ngroups = 2 * qc + 2
            es_tiles = []
            for g in range(ngroups):
                s_ps = s_pool.tile([128, 2, 512], F32, tag="s")
                es = es_pool.tile([128, 2, 512], BF16, tag="es")
                es_tiles.append(es)
                nqs = []
                for j in range(2):
                    kb = 2 * g + j
                    r = kb * 128 - qc * 512
                    q0 = max(r, 0)
                    nq = 512 - q0
                    nqs.append((j, kb, r, q0, nq))
                    nc.tensor.matmul(
                        out=s_ps[:, j, q0:512],
                        lhsT=(kt[po : po + HD, m, kb * 128 : (kb + 1) * 128]),
                        rhs=(qt[po : po + HD, m, qc * 512 + q0 : (qc + 1) * 512]),
                        start=True,
                        stop=True,
                    )
                if all(nq == 512 for (_, _, _, _, nq) in nqs):
                    nc.scalar.activation(out=es[:], in_=s_ps[:], func=_EXP)
                else:
                    for j, kb, r, q0, nq in nqs:
                        nc.scalar.activation(
                            out=es[:, j, q0:512],
                            in_=s_ps[:, j, q0:512],
                            func=_EXP,
                        )
                for j, kb, r, q0, nq in nqs:
                    if r >= 0:
                        nc.gpsimd.tensor_mul(
                            es[:, j, q0 : q0 + 128],
                            es[:, j, q0 : q0 + 128],
                            tri[:],
                        )
                # PE filler while the exp chain runs on Act
                pop_fillers(2500)
                slot_state["idx"] += 1
                if ph1q and every and slot_state["idx"] % every == 0:
                    ph1q.popleft()()
            make_av_units(h, qc, es_tiles, oT)

        # qc0/qc1 interleave with phase-1 half-0; their attention needs only
        # tokens 0:512 / 0:1024 which each pair's n-chunk unit just produced
        oT0 = oT_pool.tile([128, 4, 512], BF16, tag="oT")
        for m in range(4):
            ph1_qk_unit(0, xt0, wq, qt, m, 0)
            ph1_qk_unit(0, xt0, wk, kt, m, 0)
            if m == 0:
                # wv load sits behind the first two wt loads on the scalar
                # queue; the v matmuls themselves go between the first two
                # attention heads so the PE isn't gated on it
                for kc in range(8):
                    nc.scalar.dma_start(
                        out=wv_sb[:, kc, :], in_=wv[kc * 128 : (kc + 1) * 128, :]
                    )
            attn_head(2 * m, 0, oT0)
            if m == 0:
                for tc8 in range(4):
                    ph1_v_unit(0, xt0, tc8)
            attn_head(2 * m + 1, 0, oT0)
        flush_norm()
        make_y_units(0, oT0)

        slot_state["idx"] = 0
        oT1 = oT_pool.tile([128, 4, 512], BF16, tag="oT")
        for m in range(4):
            ph1_qk_unit(0, xt0, wq, qt, m, 1)
            ph1_qk_unit(0, xt0, wk, kt, m, 1)
            if m == 0:
                for tc8 in range(4, 8):
                    ph1_v_unit(0, xt0, tc8)
            attn_head(2 * m, 1, oT1, ph1_a, 2)
            attn_head(2 * m + 1, 1, oT1, ph1_a, 2)
        flush_norm()
        make_y_units(1, oT1)
        while ph1_a:
            ph1_a.popleft()()

        slot_state["idx"] = 0
        oT2 = oT_pool.tile([128, 4, 512], BF16, tag="oT")
        for m in range(4):
            attn_head(2 * m, 2, oT2, ph1_b, 8)
            attn_head(2 * m + 1, 2, oT2, ph1_b, 8)
        flush_norm()
        make_y_units(2, oT2)
        while ph1_b:
            ph1_b.popleft()()

        oT3 = oT_pool.tile([128, 4, 512], BF16, tag="oT")
        for m in range(4):
            ph1_qk_unit(1, xt1, wq, qt, m, 1)
            ph1_qk_unit(1, xt1, wk, kt, m, 1)
            attn_head(2 * m, 3, oT3)
            attn_head(2 * m + 1, 3, oT3)
        flush_norm()
        make_y_units(3, oT3)

        # drain remaining deferred work
        pop_fillers(10**9)


_MODULE = None


def _get_module():
    global _MODULE
    if _MODULE is None:
        _MODULE = _build_module()
    return _MODULE


def _rearr_w(w):
    # [1024, 512] -> [p, mtile, chunk, m]: w[c*128+p, mt*128+m]
    return np.ascontiguousarray(
        np.asarray(w, dtype=np.float32)
        .reshape(8, 128, 4, 128)
        .transpose(1, 2, 0, 3)
    )


def _make_in_maps(x, W_qkv, W_proj, b_proj):
    import ml_dtypes

    bf16 = ml_dtypes.bfloat16
    scale = np.float32(1.0 / np.sqrt(HD))
    bias_half = (np.asarray(b_proj, dtype=np.float32) * 0.5).reshape(1, EMB)
    in_maps = []
    for c in range(NCORES):
        b, hg = c // 2, c % 2
        cols = slice(hg * CD, (hg + 1) * CD)
        in_maps.append(
            {
                "xT": np.ascontiguousarray(
                    np.asarray(x[b], dtype=np.float32).T
                ).astype(bf16),
                "wq": _rearr_w(W_qkv[:, 0:EMB][:, cols] * scale).astype(bf16),
                "wk": _rearr_w(W_qkv[:, EMB : 2 * EMB][:, cols]).astype(bf16),
                "wv": np.ascontiguousarray(W_qkv[:, 2 * EMB : 3 * EMB][:, cols]).astype(
                    bf16
                ),
                "wp": np.ascontiguousarray(W_proj[cols, :]).astype(bf16),
                "bias": bias_half,
            }
        )
    return in_maps


def kernel(x, W_qkv, W_proj, b_proj, _trace=False, _trace_kwargs=None):
    x = np.asarray(x, dtype=np.float32)
    W_qkv = np.asarray(W_qkv, dtype=np.float32)
    W_proj = np.asarray(W_proj, dtype=np.float32)
    b_proj = np.asarray(b_proj, dtype=np.float32)

    nc = _get_module()
    in_maps = _make_in_maps(x, W_qkv, W_proj, b_proj)
    res = run_bass_kernel_spmd(
        nc, in_maps, list(range(NCORES)), trace=_trace, **(_trace_kwargs or {})
    )
    out = np.empty((B, S, EMB), dtype=np.float32)
    for b in range(B):
        out[b] = res.results[2 * b]["y"].astype(np.float32) + res.results[
            2 * b + 1
        ]["y"].astype(np.float32)
    if _trace:
        return out, res
    return out


# revision 28
# speedup vs baseline: 1.0816x; 1.0023x over previous
"""Causal attention block (B=4, S=2048, D=1024, H=16) on 8 Trainium2 NeuronCores.

Sharding: core c = (batch b = c//2, head-group hg = c%2 of 8 heads).
Each core computes QKV projection for its batch restricted to its heads'
columns, causal attention for its 8 heads, and a partial output projection
(its heads' rows of W_proj). Host sums the two partial outputs per batch
pair and returns the full [4, 2048, 1024] result.

v2 design (cost model: matmul = moving-rows only; DVE/Act = free-size only):
  - Everything bf16 (matmul speed identical to fp32r>=256, halves DMA/SBUF,
    removes the fp32r <256-moving-dim penalty so diagonal blocks need no pad).
  - Scores computed transposed sT[k, q] per 2-key-block group into a PSUM
    group tile [128, 2, 512]; ONE exp instruction per full group (Act cost
    is per-free-element, so fewer/larger activations cut the fixed 185ns
    per-instruction overhead); staircase (diagonal) blocks get exact-region
    exps. exp with no max subtraction (scores ~N(0,1), fp32 exp safe).
  - AV restructured: es is the STATIONARY operand (lhsT [128 keys, 128 q])
    and v streams (65 rows incl. a ones column for the denominator), giving
    o[q, v] naturally oriented at ~2x fewer PE rows than streaming scores.
  - Normalization in natural orientation: one batched DVE reciprocal per
    (head, qc) of the 4 denominator columns, then per-qj DVE tensor_scalar
    multiplies (per-partition scalar broadcast; no DRAM-bounce broadcast).
  - Head pairs share an o_norm [128 q, 128] tile (even head cols 0:64, odd
    64:128); one DMA-engine transpose (bf16 xbar, 14ns/tile) per qj lands
    both heads directly into the output-projection lhsT layout. No PE
    transposes, no PSUM->SBUF staging copies.
  - Output projection y = oT.T @ wp in bf16, bias added on DVE, y stored
    bf16 (host upcasts and sums the two partial cores).
  - Software pipelining: attention for qc=0,1 (which only needs tokens
    0:1024 of q/k/v) is interleaved with phase-1 half-1 QKV projection;
    AV/normalize/y work is deferred into later heads' QK/exp slots via a
    budgeted filler queue so the PE never idles while Act runs exps.
"""

from collections import deque

import numpy as np

import concourse.bass as bass
import concourse.mybir as mybir
import concourse.tile as tile
from concourse import bacc
from concourse.bass_utils import run_bass_kernel_spmd
from concourse.masks import make_upper_triangular

F32 = mybir.dt.float32
BF16 = mybir.dt.bfloat16
EMB = 1024
HEADS = 16
HD = 64
B = 4
S = 2048
NCORES = 8
HPC = 8           # heads per core
CD = HPC * HD     # 512 cols per core for each of q/k/v
NKB = S // 128    # 16 key blocks
NQC = S // 512    # 4 query chunks

_EXP = mybir.ActivationFunctionType.Exp


def _build_module():
    nc = bacc.Bacc("TRN2", target_bir_lowering=False, debug=False)
    xT = nc.declare_dram_parameter("xT", [EMB, S], BF16, isOutput=False)
    # wq/wk host-rearranged to [p, mtile, chunk, m] so each mtile loads as
    # one contiguous 2KB-per-partition DMA (128 descriptors, not 1024)
    wq = nc.declare_dram_parameter("wq", [128, 4, 8, 128], BF16, isOutput=False)
    wk = nc.declare_dram_parameter("wk", [128, 4, 8, 128], BF16, isOutput=False)
    wv = nc.declare_dram_parameter("wv", [EMB, CD], BF16, isOutput=False)
    wp = nc.declare_dram_parameter("wp", [CD, EMB], BF16, isOutput=False)
    bias = nc.declare_dram_parameter("bias", [1, EMB], F32, isOutput=False)
    y = nc.declare_dram_parameter("y", [S, EMB], BF16, isOutput=True)

    with tile.TileContext(nc) as tc:
        _body(tc, nc, xT, wq, wk, wv, wp, bias, y)
    nc.compile()
    return nc


def _body(tc, nc, xT, wq, wk, wv, wp, bias, y):
    from contextlib import ExitStack

    with ExitStack() as ctx:
        persist = ctx.enter_context(tc.tile_pool(name="persist", bufs=1))
        qt = persist.tile([128, 4, S], BF16, tag="qt")
        kt = persist.tile([128, 4, S], BF16, tag="kt")
        vx = persist.tile([128, NKB, HPC, HD + 1], BF16, tag="vx")

        # ones column for denominators (bf16 memset works; f32r did not)
        nc.gpsimd.memset(vx[:, :, :, HD : HD + 1], 1.0)
        # causal mask for diagonal blocks: tri[p, f] = 1.0 iff f >= p
        tri = persist.tile([128, 128], BF16, tag="tri")
        make_upper_triangular(nc, tri[:], val=1.0, diag=True)

        wp_sb = persist.tile([128, 4, EMB], BF16, tag="wp")
        bias_sb = persist.tile([128, 1, EMB], F32, tag="bias")
        wq_sb = persist.tile([128, 4, 8, 128], BF16, tag="wq")
        wk_sb = persist.tile([128, 4, 8, 128], BF16, tag="wk")

        # ---------------- pools ----------------
        # PSUM budget (8 banks): qkv 2 + s 2x2 + o 1 + y 1 = 8
        qkv_ps = ctx.enter_context(tc.tile_pool(name="qkvps", bufs=2, space="PSUM"))
        s_pool = ctx.enter_context(tc.tile_pool(name="sps", bufs=2, space="PSUM"))
        o_pool = ctx.enter_context(tc.tile_pool(name="ops", bufs=1, space="PSUM"))
        y_pool = ctx.enter_context(tc.tile_pool(name="yps", bufs=1, space="PSUM"))

        xt_pool = ctx.enter_context(tc.tile_pool(name="xt", bufs=2))
        wv_pool = ctx.enter_context(tc.tile_pool(name="wvp", bufs=1))
        # qc3 heads hold 8 es tiles each and the deferred AV of head h pops
        # up to two heads later: keep 3 heads' worth of buffers alive
        es_pool = ctx.enter_context(tc.tile_pool(name="es", bufs=24))
        on_pool = ctx.enter_context(tc.tile_pool(name="onorm", bufs=2))
        oT_pool = ctx.enter_context(tc.tile_pool(name="oT", bufs=3))
        ysb_pool = ctx.enter_context(tc.tile_pool(name="ysb", bufs=2))
        r_pool = ctx.enter_context(tc.tile_pool(name="recip", bufs=2))

        wv_sb = wv_pool.tile([128, 8, CD], BF16, tag="wv")
        for mm in range(4):
            nc.scalar.dma_start(out=wq_sb[:, mm], in_=wq[:, mm])
            nc.scalar.dma_start(out=wk_sb[:, mm], in_=wk[:, mm])
            if mm == 0:
                for kc in range(8):
                    nc.scalar.dma_start(
                        out=wv_sb[:, kc, :], in_=wv[kc * 128 : (kc + 1) * 128, :]
                    )

        # ---------------- phase 1 helpers ----------------
        def ph1_load_xt(half, xt_sb):
            t0 = half * 1024
            for n2 in range(2):
                for kc in range(8):
                    c0 = t0 + n2 * 512
                    # split the startup-gating loads across two queues
                    eng = nc.sync
                    eng.dma_start(
                        out=xt_sb[:, kc, n2 * 512 : (n2 + 1) * 512],
                        in_=xT[kc * 128 : (kc + 1) * 128, c0 : c0 + 512],
                    )

        def ph1_qk_unit(half, xt_sb, wdram, dst, mm, n):
            t0 = half * 1024
            wt = wq_sb if wdram is wq else wk_sb
            ps = qkv_ps.tile([128, 512], F32, tag="qkvps")
            for kc in range(8):
                nc.tensor.matmul(
                    ps[:],
                    lhsT=(wt[:, mm, kc, :]),
                    rhs=(xt_sb[:, kc, n * 512 : (n + 1) * 512]),
                    start=(kc == 0),
                    stop=(kc == 7),
                )
            col = t0 + n * 512
            nc.vector.tensor_copy(out=dst[:, mm, col : col + 512], in_=ps[:])

        def ph1_v_unit(half, xt_sb, tc8):
            tg = half * 8 + tc8
            ps = qkv_ps.tile([128, 512], F32, tag="qkvps")
            for kc in range(8):
                nc.tensor.matmul(
                    ps[:],
                    lhsT=(xt_sb[:, kc, tc8 * 128 : (tc8 + 1) * 128]),
                    rhs=(wv_sb[:, kc, :]),
                    start=(kc == 0),
                    stop=(kc == 7),
                )
            nc.vector.tensor_copy(
                out=vx[:, tg, :, 0:HD],
                in_=ps[:].rearrange("p (h d) -> p h d", h=HPC),
            )

        # ---------------- phase 1 DMA staging ----------------
        xt0 = xt_pool.tile([128, 8, 1024], BF16, tag="xt")
        ph1_load_xt(0, xt0)
        # wp/bias are not needed until the first output-projection piece
        nc.gpsimd.dma_start(
            out=wp_sb[:], in_=wp[:].rearrange("(c p) e -> p c e", p=128)
        )
        nc.gpsimd.dma_start(out=bias_sb[:], in_=bias[:].partition_broadcast(128))
        xt1 = xt_pool.tile([128, 8, 1024], BF16, tag="xt")
        ph1_load_xt(1, xt1)

        # half-1 units, paced into qc1/qc2 attention slots. Constraints:
        #  - qk n0 (tokens 1024-1535) and v kb8-11 feed qc2's QK/AV
        #    -> must fully drain during qc1
        #  - v kb12-15 feed qc3's AV -> drain during qc2
        #  - qk n1 (tokens 1536-2047) feed qc3's QK -> emitted eagerly at the
        #    top of each qc3 pair (the PE there is otherwise Act-bound)
        ph1_a = deque()
        ph1_b = deque()
        for wdram, dst in ((wq, qt), (wk, kt)):
            for mm in range(4):
                ph1_a.append(
                    lambda w=wdram, d=dst, m=mm: ph1_qk_unit(1, xt1, w, d, m, 0)
                )
        for tc8 in range(4):
            ph1_a.append(lambda t=tc8: ph1_v_unit(1, xt1, t))
        for tc8 in range(4, 8):
            ph1_b.append(lambda t=tc8: ph1_v_unit(1, xt1, t))

        # ---------------- attention ----------------
        # filler queues: (pe_rows_estimate, emit_fn). AV/norm units are
        # latency-critical (es/o_ps buffer recycling waits on them), so they
        # drain before the bulky output-projection pieces.
        fillers = deque()
        fillers_lo = deque()

        def pop_fillers(budget_rows, lo_ok=True):
            while fillers and budget_rows > 0:
                rows, fn = fillers.popleft()
                fn()
                budget_rows -= rows
            while lo_ok and fillers_lo and budget_rows > 0:
                rows, fn = fillers_lo.popleft()
                fn()
                budget_rows -= rows

        o_norm_tiles = {}
        # the normalize unit of head h is delayed until head h+1's units are
        # pushed, so by the time it pops the AV it waits on has executed and
        # the in-order DVE queue never blocks phase-1 copies behind it
        pending_norm = [None]

        def make_av_units(h, qc, es_tiles, oT):
            """AV sweep + normalize units for (h, qc). es_tiles[g] holds kb
            (2g, 2g+1). Deferred: they pop during the NEXT head's QK/exp."""
            m, e = h // 2, h % 2
            state = {"oT_tile": oT}

            def av_open():
                state["o_ps"] = o_pool.tile([128, 4, HD + 1], F32, tag="ops", name="o_ps")

            def av_qj(qj):
                o_ps = state["o_ps"]
                kb_last = 4 * qc + qj
                for kb in range(kb_last + 1):
                    g, j = kb // 2, kb % 2
                    nc.tensor.matmul(
                        out=o_ps[:, qj, :],
                        lhsT=(es_tiles[g][:, j, qj * 128 : (qj + 1) * 128]),
                        rhs=(vx[:, kb, h, :]),
                        start=(kb == 0),
                        stop=(kb == kb_last),
                    )

            def av_norm():
                o_ps = state["o_ps"]
                recip = r_pool.tile([128, 4], F32, tag="recip")
                nc.vector.reciprocal(recip[:], o_ps[:, :, HD])
                if e == 0:
                    o_norm_tiles[m] = on_pool.tile([128, 4, 128], BF16, tag="onorm", name="o_norm")
                o_norm = o_norm_tiles[m]
                for qj in range(4):
                    nc.vector.tensor_scalar_mul(
                        o_norm[:, qj, e * HD : (e + 1) * HD],
                        o_ps[:, qj, 0:HD],
                        recip[:, qj : qj + 1],
                    )
                if e == 1:
                    oT = state["oT_tile"]
                    for qj in range(4):
                        nc.sync.dma_start_transpose(
                            out=oT[:, m, qj * 128 : (qj + 1) * 128],
                            in_=o_norm[:, qj, :],
                        )

            def unit01():
                av_open()
                av_qj(0)
                av_qj(1)

            def unit23():
                av_qj(2)
                av_qj(3)

            rows01 = (4 * qc + 1 + 4 * qc + 2) * (HD + 1)
            rows23 = (4 * qc + 3 + 4 * qc + 4) * (HD + 1)
            if pending_norm[0] is not None:
                fillers.append((100, pending_norm[0]))
            fillers.append((rows01, unit01))
            fillers.append((rows23, unit23))
            pending_norm[0] = av_norm

        def flush_norm():
            if pending_norm[0] is not None:
                fillers.append((100, pending_norm[0]))
                pending_norm[0] = None

        def make_y_units(qc, oT):
            def y_piece(tc4, ncol):
                row = qc * 512 + tc4 * 128
                # late chunks alternate two PSUM banks (qkv pool is free by
                # then) so the drain pipeline doesn't serialize on one bank
                if qc >= 2 and (2 * tc4 + ncol) % 2:
                    y_ps = qkv_ps.tile([128, 512], F32, tag="qkvps", name="y_ps")
                else:
                    y_ps = y_pool.tile([128, 512], F32, tag="y", name="y_ps")
                for kc in range(4):
                    nc.tensor.matmul(
                        y_ps[:],
                        lhsT=(oT[:, kc, tc4 * 128 : (tc4 + 1) * 128]),
                        rhs=(wp_sb[:, kc, ncol * 512 : (ncol + 1) * 512]),
                        start=(kc == 0),
                        stop=(kc == 3),
                    )
                y_sb = ysb_pool.tile([128, 512], BF16, tag="ysb")
                nc.vector.tensor_add(
                    y_sb[:],
                    y_ps[:],
                    bias_sb[:, 0, ncol * 512 : (ncol + 1) * 512],
                )
                nc.sync.dma_start(
                    out=y[row : row + 128, ncol * 512 : (ncol + 1) * 512],
                    in_=y_sb[:],
                )

            for tc4 in range(4):
                for ncol in range(2):
                    fillers_lo.append(
                        (2048, lambda t=tc4, n=ncol: y_piece(t, n))
                    )

        # main loop
        slot_state = {"idx": 0}

        def attn_head(h, qc, oT, ph1q=None, every=0):
            m, e = h // 2, h % 2
            po = e * HD
            ngroups = 2 * qc + 2
            es_tiles = []
            for g in range(ngroups):
                s_ps = s_pool.tile([128, 2, 512], F32, tag="s")
                es = es_pool.tile([128, 2, 512], BF16, tag="es")
                es_tiles.append(es)
                nqs = []
                for j in range(2):
                    kb = 2 * g + j
                    r = kb * 128 - qc * 512
                    q0 = max(r, 0)
                    nq = 512 - q0
                    nqs.append((j, kb, r, q0, nq))
                    nc.tensor.matmul(
                        out=s_ps[:, j, q0:512],
                        lhsT=(kt[po : po + HD, m, kb * 128 : (kb + 1) * 128]),
                        rhs=(qt[po : po + HD, m, qc * 512 + q0 : (qc + 1) * 512]),
                        start=True,
                        stop=True,
                    )
                if all(nq == 512 for (_, _, _, _, nq) in nqs):
                    nc.scalar.activation(out=es[:], in_=s_ps[:], func=_EXP)
                else:
                    for j, kb, r, q0, nq in nqs:
                        nc.scalar.activation(
                            out=es[:, j, q0:512],
                            in_=s_ps[:, j, q0:512],
                            func=_EXP,
                        )
                for j, kb, r, q0, nq in nqs:
                    if r >= 0:
                        nc.gpsimd.tensor_mul(
                            es[:, j, q0 : q0 + 128],
                            es[:, j, q0 : q0 + 128],
                            tri[:],
                        )
                # PE filler while the exp chain runs on Act; output
                # pieces only fill the Act-bound qc2/qc3 windows
                pop_fillers(2500, lo_ok=(qc >= 2))
                slot_state["idx"] += 1
                if ph1q and every and slot_state["idx"] % every == 0:
                    ph1q.popleft()()
            make_av_units(h, qc, es_tiles, oT)

        # qc0/qc1 interleave with phase-1 half-0; their attention needs only
        # tokens 0:512 / 0:1024 which each pair's n-chunk unit just produced
        oT0 = oT_pool.tile([128, 4, 512], BF16, tag="oT")
        for m in range(4):
            ph1_qk_unit(0, xt0, wq, qt, m, 0)
            ph1_qk_unit(0, xt0, wk, kt, m, 0)
            attn_head(2 * m, 0, oT0)
            if m == 0:
                for tc8 in range(4):
                    ph1_v_unit(0, xt0, tc8)
            attn_head(2 * m + 1, 0, oT0)
        flush_norm()
        make_y_units(0, oT0)

        slot_state["idx"] = 0
        oT1 = oT_pool.tile([128, 4, 512], BF16, tag="oT")
        for m in range(4):
            ph1_qk_unit(0, xt0, wq, qt, m, 1)
            ph1_qk_unit(0, xt0, wk, kt, m, 1)
            if m == 0:
                for tc8 in range(4, 8):
                    ph1_v_unit(0, xt0, tc8)
            attn_head(2 * m, 1, oT1, ph1_a, 2)
            attn_head(2 * m + 1, 1, oT1, ph1_a, 2)
        flush_norm()
        make_y_units(1, oT1)
        while ph1_a:
            ph1_a.popleft()()

        slot_state["idx"] = 0
        oT2 = oT_pool.tile([128, 4, 512], BF16, tag="oT")
        for m in range(4):
            attn_head(2 * m, 2, oT2, ph1_b, 8)
            attn_head(2 * m + 1, 2, oT2, ph1_b, 8)
        flush_norm()
        make_y_units(2, oT2)
        while ph1_b:
            ph1_b.popleft()()

        oT3 = oT_pool.tile([128, 4, 512], BF16, tag="oT")
        for m in range(4):
            ph1_qk_unit(1, xt1, wq, qt, m, 1)
            ph1_qk_unit(1, xt1, wk, kt, m, 1)
            attn_head(2 * m, 3, oT3)
            attn_head(2 * m + 1, 3, oT3)
        flush_norm()
        make_y_units(3, oT3)

        # drain remaining deferred work
        pop_fillers(10**9)


_MODULE = None


def _get_module():
    global _MODULE
    if _MODULE is None:
        _MODULE = _build_module()
    return _MODULE


def _rearr_w(w):
    # [1024, 512] -> [p, mtile, chunk, m]: w[c*128+p, mt*128+m]
    return np.ascontiguousarray(
        np.asarray(w, dtype=np.float32)
        .reshape(8, 128, 4, 128)
        .transpose(1, 2, 0, 3)
    )


def _make_in_maps(x, W_qkv, W_proj, b_proj):
    import ml_dtypes

    bf16 = ml_dtypes.bfloat16
    scale = np.float32(1.0 / np.sqrt(HD))
    bias_half = (np.asarray(b_proj, dtype=np.float32) * 0.5).reshape(1, EMB)
    in_maps = []
    for c in range(NCORES):
        b, hg = c // 2, c % 2
        cols = slice(hg * CD, (hg + 1) * CD)
        in_maps.append(
            {
                "xT": np.ascontiguousarray(
                    np.asarray(x[b], dtype=np.float32).T
                ).astype(bf16),
                "wq": _rearr_w(W_qkv[:, 0:EMB][:, cols] * scale).astype(bf16),
                "wk": _rearr_w(W_qkv[:, EMB : 2 * EMB][:, cols]).astype(bf16),
                "wv": np.ascontiguousarray(W_qkv[:, 2 * EMB : 3 * EMB][:, cols]).astype(
                    bf16
                ),
                "wp": np.ascontiguousarray(W_proj[cols, :]).astype(bf16),
                "bias": bias_half,
            }
        )
    return in_maps


def kernel(x, W_qkv, W_proj, b_proj, _trace=False, _trace_kwargs=None):
    x = np.asarray(x, dtype=np.float32)
    W_qkv = np.asarray(W_qkv, dtype=np.float32)
    W_proj = np.asarray(W_proj, dtype=np.float32)
    b_proj = np.asarray(b_proj, dtype=np.float32)

    nc = _get_module()
    in_maps = _make_in_maps(x, W_qkv, W_proj, b_proj)
    res = run_bass_kernel_spmd(
        nc, in_maps, list(range(NCORES)), trace=_trace, **(_trace_kwargs or {})
    )
    out = np.empty((B, S, EMB), dtype=np.float32)
    for b in range(B):
        out[b] = res.results[2 * b]["y"].astype(np.float32) + res.results[
            2 * b + 1
        ]["y"].astype(np.float32)
    if _trace:
        return out, res
    return out
